# revision 17
# baseline (speedup 1.0000x reference)
"""Trainium2 Bass kernel for nn_AdHocWolfpackGNN (gnn_message_passing).

Strategy (8 NeuronCores, data-parallel over the 16 batched graphs, 2/core):
- Host sorts each graph's edges by (dst//128 window, src) and pads each of the
  16 windows-of-128-nodes to a fixed 4480 edges (dummy edges are masked out of
  the segment sum and dropped from outputs), giving a fixed SPMD schedule.
- Edge LSTM gates are computed edge-major ([128 edges, 128 gates] PSUM tiles):
  gates = [ef|h1]^T matmul (bias folded into P2') + P2'[src] + P3[dst], where
  P2' = node_feat @ W2^T + bias (built on device, bf16 [4096, 128] in DRAM)
  and P3 = node_feat @ W3^T. P2'/P3 rows (256B) are fetched per edge with
  dma_gather (MoE fast path).
- segment_sum(e_out, dst) is computed with per-tile selection matrices
  (is_equal against a staged iota) matmul-accumulated into a per-window PSUM
  tile -- no scatter at all (dma_scatter_add races on duplicate indices).
- e_comb = sum_nodes h_agg; n_comb = sum n_out; tiny graph-level LSTM runs
  replicated after an AllGather of the per-graph reductions.
"""
import os
import numpy as np
import ml_dtypes

import concourse.bacc as bacc
import concourse.mybir as mybir
import concourse.tile as tile
from concourse.bass_utils import run_bass_kernel_spmd
from concourse.masks import make_identity

F32 = mybir.dt.float32
BF16 = mybir.dt.bfloat16
I16 = mybir.dt.int16
AF = mybir.ActivationFunctionType
OP = mybir.AluOpType
BF = ml_dtypes.bfloat16

# problem geometry (hardcoded; kernel.py must be self-contained)
B, N, E = 16, 2048, 65536
H = 32
NT, ET = B * N, B * E
NCORES = int(os.environ.get("KNC", "8"))
GPC = 2                    # graphs per core
NPC = GPC * N              # nodes per core (4096)
W = 16                     # 128-node windows per graph
WIN = 128                  # nodes per window
PAD_E = 4480               # padded edges per window (Poisson(4096)+6sigma, %128)
WTILES = PAD_E // 128      # 35 tiles per window
EPG = W * PAD_E            # padded edges per graph (71680)
EPC = GPC * EPG            # padded edges per core (143360)
TILES = EPC // 128         # 1120
SLABT = 7                  # tiles per compute slab (35 = 5*7)
SLABS = WTILES // SLABT    # 5 slabs per window
KWIN = int(os.environ.get("KWIN", str(W)))   # debug: windows per graph
KNODE = os.environ.get("KNODE", "1") == "1"  # debug: run node phase
KGATH = os.environ.get("KGATH", "1") == "1"  # debug: do dma_gathers
KSEL = os.environ.get("KSEL", "1") == "1"    # debug: sel/hag path
KSLAB = os.environ.get("KSLAB", "1") == "1"  # debug: slab elementwise
GORD = np.r_[0:32, 32:64, 96:128, 64:96]  # gate reorder (i,f,g,o)->(i,f,o,g)
G_ALL = NCORES * GPC       # total graphs in the replica group

_NC_CACHE = {}
TRACE = False          # set by test harness for neuron-profile timing
LAST_EXEC_NS = [None]  # exec_time_ns of the most recent traced run


def _build_nc():
    nc = bacc.Bacc(None, target_bir_lowering=False, debug=False, num_devices=NCORES,
                   dynamic_dma_scratch_size=int(os.environ.get("KSCRATCH", "49152")))

    # ---- parameters (per-core shards staged by host) ----
    P = {}
    P["efh1T"] = nc.declare_dram_parameter("efh1T", [64, EPC], BF16, isOutput=False)
    P["h2pm"] = nc.declare_dram_parameter("h2pm", [128, TILES, 32], F32, isOutput=False)
    P["srcidx"] = nc.declare_dram_parameter("srcidx", [128, EPC // 16], I16, isOutput=False)
    P["dstidx"] = nc.declare_dram_parameter("dstidx", [128, EPC // 16], I16, isOutput=False)
    P["dstloc"] = nc.declare_dram_parameter("dstloc", [128, TILES], F32, isOutput=False)
    P["iota"] = nc.declare_dram_parameter("iota", [128, 128], BF16, isOutput=False)
    P["nfT"] = nc.declare_dram_parameter("nfT", [33, NPC], BF16, isOutput=False)
    P["w2ext"] = nc.declare_dram_parameter("w2ext", [33, 256], BF16, isOutput=False)
    P["w3"] = nc.declare_dram_parameter("w3", [32, 128], BF16, isOutput=False)
    P["wlocal"] = nc.declare_dram_parameter("wlocal", [64, 128], BF16, isOutput=False)
    P["wnode"] = nc.declare_dram_parameter("wnode", [65, 256], BF16, isOutput=False)
    P["wnhag"] = nc.declare_dram_parameter("wnhag", [32, 256], F32, isOutput=False)
    P["nh1T"] = nc.declare_dram_parameter("nh1T", [32, NPC], BF16, isOutput=False)
    P["nh2pm"] = nc.declare_dram_parameter("nh2pm", [128, 32, 32], F32, isOutput=False)
    P["wu"] = nc.declare_dram_parameter("wu", [128, 128], F32, isOutput=False)
    P["ubias"] = nc.declare_dram_parameter("ubias", [1, 128], F32, isOutput=False)
    P["guT"] = nc.declare_dram_parameter("guT", [32, G_ALL], F32, isOutput=False)
    P["gh1T"] = nc.declare_dram_parameter("gh1T", [32, G_ALL], F32, isOutput=False)
    P["gh2"] = nc.declare_dram_parameter("gh2", [G_ALL, 32], F32, isOutput=False)

    O = {}
    for nm in ("eh", "ec", "eo"):
        O[nm] = nc.declare_dram_parameter(nm, [128, TILES, 32], F32, isOutput=True)
    for nm in ("nh", "ncell", "no"):
        O[nm] = nc.declare_dram_parameter(nm, [128, 32, 32], F32, isOutput=True)
    for nm in ("uh", "uc", "uo"):
        O[nm] = nc.declare_dram_parameter(nm, [G_ALL, 32], F32, isOutput=True)

    p2 = nc.dram_tensor("p2_tab", [NPC, 128], F32)
    p3 = nc.dram_tensor("p3_tab", [NPC, 128], F32)
    agin = nc.dram_tensor("ag_in", [2, 64], F32)
    agout = nc.dram_tensor("ag_out", [G_ALL, 64], F32,
                           addr_space="Shared" if NCORES > 4 else "Local")

    from contextlib import ExitStack
    with tile.TileContext(nc) as tc, ExitStack() as ctx:
        cst = ctx.enter_context(tc.tile_pool(name="cst", bufs=1))
        io = ctx.enter_context(tc.tile_pool(name="io", bufs=2))
        wk = ctx.enter_context(tc.tile_pool(name="wk", bufs=2))
        pgate = ctx.enter_context(tc.tile_pool(name="pgate", bufs=2, space="PSUM"))
        phag = ctx.enter_context(tc.tile_pool(name="phag", bufs=2, space="PSUM"))
        psml = ctx.enter_context(tc.tile_pool(name="psml", bufs=2, space="PSUM"))

        # ---- persistent constants ----
        iota_s = cst.tile([128, 128], BF16, tag="iota")
        nc.sync.dma_start(out=iota_s[:], in_=P["iota"][:])
        wloc_s = cst.tile([64, 128], BF16, tag="wloc")
        nc.sync.dma_start(out=wloc_s[:], in_=P["wlocal"][:])
        nfT_s = cst.tile([33, NPC], BF16, tag="nfT")
        nc.sync.dma_start(out=nfT_s[:], in_=P["nfT"][:])
        w2_s = cst.tile([33, 256], BF16, tag="w2")
        nc.sync.dma_start(out=w2_s[:], in_=P["w2ext"][:])
        w3_s = cst.tile([32, 128], BF16, tag="w3")
        nc.sync.dma_start(out=w3_s[:], in_=P["w3"][:])
        wn_s = cst.tile([65, 256], BF16, tag="wn")
        nc.sync.dma_start(out=wn_s[:], in_=P["wnode"][:])
        wnh_s = cst.tile([32, 256], F32, tag="wnh")
        nc.sync.dma_start(out=wnh_s[:], in_=P["wnhag"][:])
        nh1T_s = cst.tile([32, NPC], BF16, tag="nh1T")
        nc.sync.dma_start(out=nh1T_s[:], in_=P["nh1T"][:])
        wu_s = cst.tile([128, 128], F32, tag="wu")
        nc.sync.dma_start(out=wu_s[:], in_=P["wu"][:])
        ub_s = cst.tile([1, 128], F32, tag="ub")
        nc.sync.dma_start(out=ub_s[:], in_=P["ubias"][:])
        guT_s = cst.tile([32, G_ALL], F32, tag="guT")
        nc.sync.dma_start(out=guT_s[:], in_=P["guT"][:])
        gh1T_s = cst.tile([32, G_ALL], F32, tag="gh1T")
        nc.sync.dma_start(out=gh1T_s[:], in_=P["gh1T"][:])
        gh2_s = cst.tile([G_ALL, 32], F32, tag="gh2")
        nc.sync.dma_start(out=gh2_s[:], in_=P["gh2"][:])
        ident_s = cst.tile([128, 128], F32, tag="ident")
        make_identity(nc, ident_s[:])
        ones128_s = cst.tile([128, 1], F32, tag="ones128")
        nc.gpsimd.memset(ones128_s[:], 1.0)
        ones16_s = cst.tile([1, G_ALL], F32, tag="ones16")
        nc.gpsimd.memset(ones16_s[:], 1.0)

        # per-graph persistent tiles
        xtn_s = [cst.tile([65, N], BF16, tag=f"xtn{g}", name=f"xtn{g}") for g in range(GPC)]
        hagT_s = [cst.tile([32, N], F32, tag=f"hagT{g}", name=f"hagT{g}") for g in range(GPC)]
        ecomb_s = [cst.tile([128, 32], F32, tag=f"ecomb{g}", name=f"ecomb{g}") for g in range(GPC)]
        ncomb_s = [cst.tile([128, 32], F32, tag=f"ncomb{g}", name=f"ncomb{g}") for g in range(GPC)]
        for g in range(GPC):
            nc.gpsimd.memset(ecomb_s[g][:], 0.0)
            nc.gpsimd.memset(ncomb_s[g][:], 0.0)
            # node-phase lhsT: rows 0-31 nf^T, 64-95 node h1^T, row 96 ones
            nc.vector.tensor_copy(out=xtn_s[g][0:32, :], in_=nfT_s[0:32, g * N:(g + 1) * N])
            nc.vector.tensor_copy(out=xtn_s[g][32:64, :], in_=nh1T_s[:, g * N:(g + 1) * N])
            nc.gpsimd.memset(xtn_s[g][64:65, :], 1.0)

        # ---- build P2' (with per-graph bias) and P3 tables in DRAM ----
        for t in range(NPC // 128):
            g = t // (N // 128)
            pp = psml.tile([128, 128], F32, tag="psmall", space="PSUM")
            nc.tensor.matmul(out=pp[:], lhsT=nfT_s[:, t * 128:(t + 1) * 128],
                             rhs=w2_s[:, g * 128:(g + 1) * 128], start=True, stop=True)
            pb = wk.tile([128, 128], F32, tag="pbuild_sb")
            nc.vector.tensor_copy(out=pb[:], in_=pp[:])
            nc.sync.dma_start(out=p2[t * 128:(t + 1) * 128, :], in_=pb[:])
            pp3 = psml.tile([128, 128], F32, tag="psmall", space="PSUM")
            nc.tensor.matmul(out=pp3[:], lhsT=nfT_s[0:32, t * 128:(t + 1) * 128],
                             rhs=w3_s[:], start=True, stop=True)
            pb3 = wk.tile([128, 128], F32, tag="pbuild_sb")
            nc.vector.tensor_copy(out=pb3[:], in_=pp3[:])
            nc.sync.dma_start(out=p3[t * 128:(t + 1) * 128, :], in_=pb3[:])

        # ---- edge phase: per graph, per 128-node window ----
        for g in range(GPC):
            for w in range(KWIN):
                wi = g * W + w           # global window index
                T0 = wi * WTILES         # first tile of window
                E0 = wi * PAD_E          # first edge of window

                efh1 = io.tile([64, PAD_E], BF16, tag="efh1")
                nc.sync.dma_start(out=efh1[:], in_=P["efh1T"][:, E0:E0 + PAD_E])
                h2c = io.tile([128, WTILES, 32], F32, tag="h2c")
                nc.sync.dma_start(out=h2c[:], in_=P["h2pm"][:, T0:T0 + WTILES, :])
                dloc = io.tile([128, WTILES], F32, tag="dloc")
                nc.sync.dma_start(out=dloc[:], in_=P["dstloc"][:, T0:T0 + WTILES])
                sidx = io.tile([128, PAD_E // 16], I16, tag="sidx")
                nc.sync.dma_start(out=sidx[:], in_=P["srcidx"][:, E0 // 16:(E0 + PAD_E) // 16])
                didx = io.tile([128, PAD_E // 16], I16, tag="didx")
                nc.sync.dma_start(out=didx[:], in_=P["dstidx"][:, E0 // 16:(E0 + PAD_E) // 16])

                sg = io.tile([128, WTILES, 128], F32, tag="sgath")
                dg = io.tile([128, WTILES, 128], F32, tag="dgath")
                if KGATH:
                    GT = int(os.environ.get("KGT", "7"))  # tiles per gather chunk
                    for cstart in range(0, WTILES, GT):
                        ct = min(GT, WTILES - cstart)
                        nidx = ct * 128
                        nc.gpsimd.dma_gather(
                            out_ap=sg[:, cstart:cstart + ct, :], in_ap=p2[:],
                            idxs_ap=sidx[:, cstart * 8:(cstart + ct) * 8],
                            num_idxs=nidx, num_idxs_reg=nidx, elem_size=128)
                        nc.gpsimd.dma_gather(
                            out_ap=dg[:, cstart:cstart + ct, :], in_ap=p3[:],
                            idxs_ap=didx[:, cstart * 8:(cstart + ct) * 8],
                            num_idxs=nidx, num_idxs_reg=nidx, elem_size=128)
                else:
                    nc.gpsimd.memset(sg[:], 0.0)
                    nc.gpsimd.memset(dg[:], 0.0)

                hag = phag.tile([128, 32], F32, tag="hag", space="PSUM")
                oeh = io.tile([128, WTILES, 32], F32, tag="oeh")
                oec = io.tile([128, WTILES, 32], F32, tag="oec")
                oeo = io.tile([128, WTILES, 32], F32, tag="oeo")

                for s in range(SLABS):
                    ts = s * SLABT
                    gp = pgate.tile([128, SLABT, 128], F32, tag="gates", space="PSUM")
                    for k in range(SLABT):
                        t = ts + k
                        nc.tensor.matmul(out=gp[:, k, :],
                                         lhsT=efh1[:, t * 128:(t + 1) * 128],
                                         rhs=wloc_s[:], start=True, stop=True)
                    if not KSLAB:
                        continue
                    gsb = wk.tile([128, SLABT, 128], BF16, tag="gsb")
                    nc.vector.tensor_tensor(out=gsb[:], in0=gp[:], in1=sg[:, ts:ts + SLABT, :], op=OP.add)
                    nc.vector.tensor_tensor(out=gsb[:], in0=gsb[:], in1=dg[:, ts:ts + SLABT, :], op=OP.add)
                    sig = wk.tile([128, SLABT, 96], F32, tag="sig")
                    nc.scalar.activation(out=sig[:], in_=gsb[:, :, 0:96], func=AF.Sigmoid)
                    tg = wk.tile([128, SLABT, 32], F32, tag="tg")
                    nc.scalar.activation(out=tg[:], in_=gsb[:, :, 96:128], func=AF.Tanh)
                    t1 = wk.tile([128, SLABT, 32], F32, tag="t1")
                    nc.vector.tensor_tensor(out=t1[:], in0=sig[:, :, 32:64],
                                            in1=h2c[:, ts:ts + SLABT, :], op=OP.mult)
                    t2 = wk.tile([128, SLABT, 32], F32, tag="t2")
                    nc.vector.tensor_tensor(out=t2[:], in0=sig[:, :, 0:32], in1=tg[:], op=OP.mult)
                    nc.vector.tensor_tensor(out=oec[:, ts:ts + SLABT, :], in0=t1[:], in1=t2[:], op=OP.add)
                    tc2 = wk.tile([128, SLABT, 32], F32, tag="tc2")
                    nc.scalar.activation(out=tc2[:], in_=oec[:, ts:ts + SLABT, :], func=AF.Tanh)
                    nc.vector.tensor_tensor(out=oeh[:, ts:ts + SLABT, :], in0=sig[:, :, 64:96],
                                            in1=tc2[:], op=OP.mult)
                    nc.scalar.activation(out=oeo[:, ts:ts + SLABT, :], in_=oeh[:, ts:ts + SLABT, :],
                                         func=AF.Relu)
                    eob = wk.tile([128, SLABT, 32], BF16, tag="eob")
                    nc.vector.tensor_copy(out=eob[:], in_=oeo[:, ts:ts + SLABT, :])
                    for k in range(SLABT if KSEL else 0):
                        t = ts + k
                        sel = wk.tile([128, 128], BF16, tag="sel")
                        nc.vector.tensor_scalar(out=sel[:], in0=iota_s[:],
                                                scalar1=dloc[:, t:t + 1], scalar2=None,
                                                op0=OP.is_equal)
                        nc.tensor.matmul(out=hag[:], lhsT=sel[:], rhs=eob[:, k, :],
                                         start=(t == 0), stop=(t == WTILES - 1))

                if not KSLAB:
                    continue
                nc.sync.dma_start(out=O["eh"][:, T0:T0 + WTILES, :], in_=oeh[:])
                nc.sync.dma_start(out=O["ec"][:, T0:T0 + WTILES, :], in_=oec[:])
                nc.sync.dma_start(out=O["eo"][:, T0:T0 + WTILES, :], in_=oeo[:])

                # e_comb accumulation and h_agg^T for the node phase
                nc.vector.tensor_tensor(out=ecomb_s[g][:], in0=ecomb_s[g][:], in1=hag[:], op=OP.add)
                hcb = wk.tile([128, 32], F32, tag="hcb")
                nc.vector.tensor_copy(out=hcb[:], in_=hag[:])
                for b in range(4):
                    nc.vector.transpose(out=hagT_s[g][:, w * 128 + b * 32:w * 128 + (b + 1) * 32],
                                        in_=hcb[b * 32:(b + 1) * 32, :])

        # ---- node phase ----
        for g in range(GPC):
            for t in range(N // 128 if KNODE else 0):
                npp = psml.tile([128, 128], F32, tag="psmall", space="PSUM")
                nc.tensor.matmul(out=npp[:], lhsT=xtn_s[g][:, t * 128:(t + 1) * 128],
                                 rhs=wn_s[:, g * 128:(g + 1) * 128], start=True, stop=False)
                nc.tensor.matmul(out=npp[:], lhsT=hagT_s[g][:, t * 128:(t + 1) * 128],
                                 rhs=wnh_s[:, g * 128:(g + 1) * 128], start=False, stop=True)
                gt = g * (N // 128) + t
                sign = wk.tile([128, 96], F32, tag="sign")
                nc.scalar.activation(out=sign[:], in_=npp[:, 0:96], func=AF.Sigmoid)
                tgn = wk.tile([128, 32], F32, tag="tgn")
                nc.scalar.activation(out=tgn[:], in_=npp[:, 96:128], func=AF.Tanh)
                nh2 = wk.tile([128, 32], F32, tag="nh2")
                nc.sync.dma_start(out=nh2[:], in_=P["nh2pm"][:, gt, :])
                t1n = wk.tile([128, 32], F32, tag="t1n")
                nc.vector.tensor_tensor(out=t1n[:], in0=sign[:, 32:64], in1=nh2[:], op=OP.mult)
                t2n = wk.tile([128, 32], F32, tag="t2n")
                nc.vector.tensor_tensor(out=t2n[:], in0=sign[:, 0:32], in1=tgn[:], op=OP.mult)
                occ = wk.tile([128, 32], F32, tag="occ")
                nc.vector.tensor_tensor(out=occ[:], in0=t1n[:], in1=t2n[:], op=OP.add)
                nc.sync.dma_start(out=O["ncell"][:, gt, :], in_=occ[:])
                tcn = wk.tile([128, 32], F32, tag="tcn")
                nc.scalar.activation(out=tcn[:], in_=occ[:], func=AF.Tanh)
                ohn = wk.tile([128, 32], F32, tag="ohn")
                nc.vector.tensor_tensor(out=ohn[:], in0=sign[:, 64:96], in1=tcn[:], op=OP.mult)
                nc.sync.dma_start(out=O["nh"][:, gt, :], in_=ohn[:])
                oon = wk.tile([128, 32], F32, tag="oon")
                nc.scalar.activation(out=oon[:], in_=ohn[:], func=AF.Relu)
                nc.sync.dma_start(out=O["no"][:, gt, :], in_=oon[:])
                nc.vector.tensor_tensor(out=ncomb_s[g][:], in0=ncomb_s[g][:], in1=oon[:], op=OP.add)

        # ---- graph reductions + AllGather ----
        for g in range(GPC):
            rp = psml.tile([1, 64], F32, tag="psmall", space="PSUM")
            nc.tensor.matmul(out=rp[:, 0:32], lhsT=ones128_s[:], rhs=ncomb_s[g][:],
                             start=True, stop=True)
            nc.tensor.matmul(out=rp[:, 32:64], lhsT=ones128_s[:], rhs=ecomb_s[g][:],
                             start=True, stop=True)
            rsb = wk.tile([1, 64], F32, tag="rsb")
            nc.vector.tensor_copy(out=rsb[:], in_=rp[:])
            nc.sync.dma_start(out=agin[g:g + 1, :], in_=rsb[:])
        nc.gpsimd.collective_compute(
            "AllGather", OP.bypass, replica_groups=[list(range(NCORES))],
            ins=[agin[:]], outs=[agout[:]])
        agsb = wk.tile([G_ALL, 64], F32, tag="agsb")
        nc.sync.dma_start(out=agsb[:], in_=agout[:])

        # ---- graph-level LSTM (replicated on every core) ----
        tp = psml.tile([64, G_ALL], F32, tag="psmall", space="PSUM")
        nc.tensor.transpose(out=tp[:], in_=agsb[:], identity=ident_s[0:G_ALL, 0:G_ALL])
        xtu = wk.tile([128, G_ALL], F32, tag="xtu")
        nc.vector.tensor_copy(out=xtu[0:64, :], in_=tp[:])
        nc.vector.tensor_copy(out=xtu[64:96, :], in_=guT_s[:])
        nc.vector.tensor_copy(out=xtu[96:128, :], in_=gh1T_s[:])
        up = psml.tile([G_ALL, 128], F32, tag="psmall", space="PSUM")
        nc.tensor.matmul(out=up[:], lhsT=xtu[:], rhs=wu_s[:], start=True, stop=False)
        nc.tensor.matmul(out=up[:], lhsT=ones16_s[:], rhs=ub_s[:], start=False, stop=True)
        sigu = wk.tile([G_ALL, 96], F32, tag="sigu")
        nc.scalar.activation(out=sigu[:], in_=up[:, 0:96], func=AF.Sigmoid)
        tgu = wk.tile([G_ALL, 32], F32, tag="tgu")
        nc.scalar.activation(out=tgu[:], in_=up[:, 96:128], func=AF.Tanh)
        t1u = wk.tile([G_ALL, 32], F32, tag="t1u")
        nc.vector.tensor_tensor(out=t1u[:], in0=sigu[:, 32:64], in1=gh2_s[:], op=OP.mult)
        t2u = wk.tile([G_ALL, 32], F32, tag="t2u")
        nc.vector.tensor_tensor(out=t2u[:], in0=sigu[:, 0:32], in1=tgu[:], op=OP.mult)
        ucc = wk.tile([G_ALL, 32], F32, tag="ucc")
        nc.vector.tensor_tensor(out=ucc[:], in0=t1u[:], in1=t2u[:], op=OP.add)
        nc.sync.dma_start(out=O["uc"][:], in_=ucc[:])
        tcu = wk.tile([G_ALL, 32], F32, tag="tcu")
        nc.scalar.activation(out=tcu[:], in_=ucc[:], func=AF.Tanh)
        uhh = wk.tile([G_ALL, 32], F32, tag="uhh")
        nc.vector.tensor_tensor(out=uhh[:], in0=sigu[:, 64:96], in1=tcu[:], op=OP.mult)
        nc.sync.dma_start(out=O["uh"][:], in_=uhh[:])
        uoo = wk.tile([G_ALL, 32], F32, tag="uoo")
        nc.scalar.activation(out=uoo[:], in_=uhh[:], func=AF.Relu)
        nc.sync.dma_start(out=O["uo"][:], in_=uoo[:])

    nc.compile()
    return nc


def _wrap16(idx):
    w = idx.reshape(-1, 16).T.astype(np.int16)
    return np.ascontiguousarray(np.tile(w, (8, 1)))


def _pmajor(a):
    # [EPC, d] -> [128, TILES, d]
    d = a.shape[1]
    return np.ascontiguousarray(a.reshape(-1, 128, d).transpose(1, 0, 2))


def _stage(inputs):
    """Returns (in_maps, positions) where positions[g] maps original edge ->
    padded stream position within graph g's stream."""
    ef = np.asarray(inputs["edge_feat"])
    nf = np.asarray(inputs["node_feat"])
    gr = np.asarray(inputs["g_repr"])
    eh1 = np.asarray(inputs["edge_h1"])
    eh2 = np.asarray(inputs["edge_h2"])
    nh1 = np.asarray(inputs["node_h1"])
    nh2 = np.asarray(inputs["node_h2"])
    gh1 = np.asarray(inputs["graph_h1"])
    gh2 = np.asarray(inputs["graph_h2"])
    esrc = np.asarray(inputs["edge_src"])
    edst = np.asarray(inputs["edge_dst"])

    def rw(m):  # reorder gate rows
        return np.asarray(m)[GORD]

    We = rw(inputs["We_ih"]); Weh = rw(inputs["We_hh"])
    be = rw(inputs["be_ih"]) + rw(inputs["be_hh"])
    Wn = rw(inputs["Wn_ih"]); Wnh = rw(inputs["Wn_hh"])
    bn = rw(inputs["bn_ih"]) + rw(inputs["bn_hh"])
    Wu = rw(inputs["Wu_ih"]); Wuh = rw(inputs["Wu_hh"])
    bu = rw(inputs["bu_ih"]) + rw(inputs["bu_hh"])

    wlocal = np.concatenate([We[:, 0:32].T, Weh.T], 0).astype(BF)        # [64,128]
    w3 = We[:, 64:96].T.astype(BF)                                       # [32,128]
    wu_t = np.concatenate([Wu[:, 0:32].T, Wu[:, 32:64].T, Wu[:, 64:96].T, Wuh.T], 0).astype(np.float32)
    ubias = bu.reshape(1, 128).astype(np.float32)
    guT = gr.T[:, :G_ALL].astype(np.float32)
    gh1T = gh1.T[:, :G_ALL].astype(np.float32)
    iota = np.broadcast_to(np.arange(128, dtype=np.float32), (128, 128)).astype(BF)

    positions = []
    in_maps = []
    for c in range(NCORES):
        st_ef = np.zeros((EPC, 32), np.float32)
        st_h1 = np.zeros((EPC, 32), np.float32)
        st_h2 = np.zeros((EPC, 32), np.float32)
        st_src = np.zeros(EPC, np.int16)
        st_dst = np.zeros(EPC, np.int16)
        st_dl = np.full(EPC, -1.0, np.float32)
        for gl in range(GPC):
            g = c * GPC + gl
            sl = slice(g * E, (g + 1) * E)
            src_g, dst_g = esrc[sl], edst[sl]
            wnd = dst_g // WIN
            order = np.lexsort((src_g, wnd))
            wnd_s = wnd[order]
            cnt = np.bincount(wnd_s, minlength=W)
            assert cnt.max() <= PAD_E, f"window overflow: {cnt.max()} > {PAD_E}"
            starts = np.zeros(W, np.int64)
            starts[1:] = np.cumsum(cnt)[:-1]
            rank = np.arange(E) - starts[wnd_s]
            pos_sorted = wnd_s * PAD_E + rank
            pos = np.empty(E, np.int64)
            pos[order] = pos_sorted
            positions.append(pos) if c * GPC + gl == len(positions) else None
            off = gl * EPG
            p = off + pos
            st_ef[p] = ef[sl]
            st_h1[p] = eh1[sl]
            st_h2[p] = eh2[sl]
            st_src[p] = (gl * N + src_g).astype(np.int16)
            st_dst[p] = (gl * N + dst_g).astype(np.int16)
            st_dl[p] = (dst_g - (dst_g // WIN) * WIN).astype(np.float32)

        efh1T = np.concatenate([st_ef.T, st_h1.T], 0).astype(BF)
        gsl = slice(c * GPC * N, (c + 1) * GPC * N)
        nf_c = nf[gsl]
        nfT = np.concatenate([nf_c.T, np.ones((1, NPC), np.float32)], 0).astype(BF)
        w2ext = np.zeros((33, 256), np.float32)
        wnode = np.zeros((65, 256), np.float32)
        wnhag = np.zeros((32, 256), np.float32)
        for gl in range(GPC):
            g = c * GPC + gl
            w2ext[0:32, gl * 128:(gl + 1) * 128] = We[:, 32:64].T
            w2ext[32, gl * 128:(gl + 1) * 128] = be + gr[g] @ We[:, 96:128].T
            wnode[0:32, gl * 128:(gl + 1) * 128] = Wn[:, 0:32].T
            wnode[32:64, gl * 128:(gl + 1) * 128] = Wnh.T
            wnode[64, gl * 128:(gl + 1) * 128] = bn + gr[g] @ Wn[:, 64:96].T
            wnhag[:, gl * 128:(gl + 1) * 128] = Wn[:, 32:64].T

        in_maps.append(dict(
            efh1T=efh1T,
            h2pm=_pmajor(st_h2),
            srcidx=_wrap16(st_src),
            dstidx=_wrap16(st_dst),
            dstloc=np.ascontiguousarray(st_dl.reshape(TILES, 128).T).astype(np.float32),
            iota=iota,
            nfT=nfT,
            w2ext=w2ext.astype(BF),
            w3=w3,
            wlocal=wlocal,
            wnode=wnode.astype(BF),
            wnhag=wnhag,
            nh1T=nh1[gsl].T.astype(BF),
            nh2pm=_pmajor(nh2[gsl]),
            wu=wu_t,
            ubias=ubias,
            guT=guT,
            gh1T=gh1T,
            gh2=gh2[:G_ALL].astype(np.float32),
        ))
    return in_maps, positions


def kernel(**inputs):
    if "nc" not in _NC_CACHE:
        _NC_CACHE["nc"] = _build_nc()
    nc = _NC_CACHE["nc"]
    in_maps, positions = _stage(inputs)
    res = run_bass_kernel_spmd(nc, in_maps, core_ids=list(range(NCORES)), trace=TRACE)
    LAST_EXEC_NS[0] = res.exec_time_ns
    r = res.results

    e_out = np.zeros((ET, 32), np.float32)
    e_h = np.zeros((ET, 32), np.float32)
    e_c = np.zeros((ET, 32), np.float32)
    n_out = np.zeros((NT, 32), np.float32)
    n_h = np.zeros((NT, 32), np.float32)
    n_c = np.zeros((NT, 32), np.float32)
    for c in range(NCORES):
        eo = np.asarray(r[c]["eo"]).transpose(1, 0, 2).reshape(EPC, 32)
        ehh = np.asarray(r[c]["eh"]).transpose(1, 0, 2).reshape(EPC, 32)
        ecc = np.asarray(r[c]["ec"]).transpose(1, 0, 2).reshape(EPC, 32)
        for gl in range(GPC):
            g = c * GPC + gl
            pos = positions[g] + gl * EPG
            e_out[g * E:(g + 1) * E] = eo[pos]
            e_h[g * E:(g + 1) * E] = ehh[pos]
            e_c[g * E:(g + 1) * E] = ecc[pos]
        n_out[c * NPC:(c + 1) * NPC] = np.asarray(r[c]["no"]).transpose(1, 0, 2).reshape(NPC, 32)
        n_h[c * NPC:(c + 1) * NPC] = np.asarray(r[c]["nh"]).transpose(1, 0, 2).reshape(NPC, 32)
        n_c[c * NPC:(c + 1) * NPC] = np.asarray(r[c]["ncell"]).transpose(1, 0, 2).reshape(NPC, 32)
    u_out = np.asarray(r[0]["uo"]).astype(np.float32)
    u_h = np.asarray(r[0]["uh"]).astype(np.float32)
    u_c = np.asarray(r[0]["uc"]).astype(np.float32)
    return (e_out, e_h, e_c, n_out, n_h, n_c, u_out, u_h, u_c)
